# revision 17
# baseline (speedup 1.0000x reference)
"""Fused cross-attention kernel for Trainium2 (Bass/Tile), 8-core SPMD.

Problem: query/key_value [T=4, B=2, C=128, H=32, W=32] -> tokens [B, N=4096, C],
QKV projections (128x128), full softmax attention over N tokens per batch.

Sharding: core = b*4 + t handles batch b, query tokens [t*1024, (t+1)*1024)
against all 4096 K/V tokens of batch b. QKV weights replicated.

Device layout (per core):
  qpack [C, C+1024]   [Wq^T | q_x^T] (C on partitions)
  kpack [C, 2C+4096]  [Wk^T | Wv^T | kv_x^T]
  QT = Wq^T-stationary matmuls -> [d, n];  KT -> [d, m];  V -> [m, d] natural.
  Attention streamed over m in chunks of 128 per query half of 512:
    S^T chunk  = KT_chunk.T @ QT        (psum [m=128, n=512])
    P = exp(scale * S^T)                (ACT, PSUM->SBUF, fp32r)
    O^T       += V_chunk.T @ P          (psum [d=128, n=512], PE-accumulated)
    rowsum via DVE/GPSIMD accumulator chains (or PE ones-matmuls)
  K/V projections are software-pipelined into the first half's chunk loop.
  Normalize with 1/rowsum applied per n-block after a PE transpose, DMA out.

All heavy matmuls run in fp32r (single-pass fp32, ~1.5e-4 matmul rel err,
4x faster than exact fp32 on the PE); the normalization chain stays fp32.

Bias handling: bq applied on-device to Q^T (per-partition ACT bias); bk shifts
every score of a row equally so it drops out of softmax exactly; bv is added
on the host after the gather (softmax weights sum to 1).
"""

import math
from contextlib import ExitStack

import numpy as np

import concourse.bass as bass
import concourse.mybir as mybir
import concourse.tile as tile
from concourse import bacc
from concourse.bass_utils import run_bass_kernel_spmd
from concourse.masks import make_identity

F32 = mybir.dt.float32
F32R = mybir.dt.float32r
AF = mybir.ActivationFunctionType

C = 128        # model dim
NQ = 1024      # query tokens per core
M = 4096       # kv tokens per batch
T = 4
B = 2
SCALE = 1.0 / math.sqrt(float(C))
N_CORES = 8

CFG = dict(
    sum_mode="dve",    # "dve": DVE/GPSIMD accumulator chains; "pe": ones-matmuls
    interleave=True,   # pipeline K/V projections into the h=0 chunk loop
    copies_on="act",   # engine for K/V projection PSUM->SBUF copies
    ps_s_bufs=4,       # score PSUM buffers (banks)
    p_bufs=6,          # exp output SBUF buffers
    gp_every=3,        # every gp_every-th chunk's sum-add goes to GPSIMD
    misc_bufs=2,       # ps_misc PSUM banks
)

_NC = None


def build_nc(reps=1, loop_reps=0, **overrides):
    cfg = dict(CFG)
    cfg.update(overrides)
    sum_mode = cfg["sum_mode"]
    copy_eng_name = cfg["copies_on"]

    nc = bacc.Bacc()
    qpack = nc.dram_tensor("qpack", [C, C + NQ], F32R, kind="ExternalInput")
    kpack = nc.dram_tensor("kpack", [C, 2 * C + M], F32R, kind="ExternalInput")
    bq = nc.dram_tensor("bq", [C, 1], F32, kind="ExternalInput")
    out = nc.dram_tensor("out", [NQ, C], F32, kind="ExternalOutput")

    with tile.TileContext(nc) as tc, ExitStack() as ctx:
        const = ctx.enter_context(tc.tile_pool(name="const", bufs=1))
        proj = ctx.enter_context(tc.tile_pool(name="proj", bufs=1))
        pwork = ctx.enter_context(tc.tile_pool(name="pwork", bufs=cfg["p_bufs"]))
        owork = ctx.enter_context(tc.tile_pool(name="owork", bufs=2))
        outp = ctx.enter_context(tc.tile_pool(name="outp", bufs=3))
        psum = ctx.enter_context(tc.tile_pool(name="psum", bufs=2, space="PSUM"))

        def eng_copy(dst, src):
            if copy_eng_name == "act":
                nc.scalar.copy(dst, src)
            else:
                nc.vector.tensor_copy(dst, src)

        # Constants (gpsimd/DVE, no DMA deps). Warm the exp table first.
        ones_f32 = const.tile([128, 1], F32)
        nc.gpsimd.memset(ones_f32, 1.0)
        warm = const.tile([128, 1], F32)
        nc.scalar.activation(warm, ones_f32, AF.Exp)
        ones_col = const.tile([128, 1], F32R)
        nc.vector.tensor_copy(ones_col, ones_f32)
        ones_row = const.tile([1, 128], F32)
        nc.gpsimd.memset(ones_row, 1.0)
        ident = const.tile([128, 128], F32)
        make_identity(nc, ident)

        # Input DMAs: qpack on the sync (SP) HWDGE ring, kpack on the
        # scalar (ACT) HWDGE ring so the two streams run in parallel.
        qpack_sb = const.tile([C, C + NQ], F32R)
        nc.sync.dma_start(qpack_sb[:, 0:640], qpack[:, 0:640])
        nc.sync.dma_start(qpack_sb[:, 640:C + NQ], qpack[:, 640:C + NQ])
        bq_sb = const.tile([C, 1], F32)
        nc.sync.dma_start(bq_sb, bq[:])
        kpack_sb = const.tile([C, 2 * C + M], F32R)
        nc.scalar.dma_start(kpack_sb[:, 0:1280], kpack[:, 0:1280])
        for i in range(3):
            lo, hi = 1280 + i * 1024, 2304 + i * 1024
            nc.scalar.dma_start(kpack_sb[:, lo:hi], kpack[:, lo:hi])

        wq_sb = qpack_sb[:, 0:C]
        qx_sb = qpack_sb[:, C:]
        wk_sb = kpack_sb[:, 0:C]
        wv_sb = kpack_sb[:, C:2 * C]
        kvx_sb = kpack_sb[:, 2 * C:]

        # Wv^T duplicated side by side so V-projection matmuls have N=256
        # (full fp32r rate needs moving free dim >= 256).
        wv2_sb = const.tile([C, 2 * C], F32R)
        nc.vector.tensor_copy(wv2_sb[:, 0:C], wv_sb)
        nc.vector.tensor_copy(wv2_sb[:, C:2 * C], wv_sb)

        loop_cm = tc.For_i(0, loop_reps, 1) if loop_reps else None
        if loop_cm is not None:
            loop_cm.__enter__()
        for _rep in range(reps):
            # ---- projections (Q up front; K/V optionally interleaved) ----
            qT = proj.tile([C, NQ], F32R)
            for i in range(NQ // 512):
                psq = psum.tile([128, 512], F32, tag="ps_misc", bufs=cfg["misc_bufs"])
                nc.tensor.matmul(
                    psq, lhsT=wq_sb, rhs=qx_sb[:, i * 512:(i + 1) * 512],
                    start=True, stop=True,
                )
                nc.scalar.activation(
                    qT[:, i * 512:(i + 1) * 512], psq, AF.Identity, bias=bq_sb,
                )

            kT = proj.tile([C, M], F32R)
            v_sb = proj.tile([C, M], F32R)  # V chunk j at cols [j*128, (j+1)*128)

            def emit_kproj(i):
                # kT columns [i*512, (i+1)*512)
                psk = psum.tile([128, 512], F32, tag="ps_misc", bufs=cfg["misc_bufs"],
                                name="psk")
                nc.tensor.matmul(
                    psk, lhsT=wk_sb, rhs=kvx_sb[:, i * 512:(i + 1) * 512],
                    start=True, stop=True,
                )
                eng_copy(kT[:, i * 512:(i + 1) * 512], psk)

            def emit_vproj(g):
                # V chunks 2g, 2g+1
                psv = psum.tile([128, 512], F32, tag="ps_misc", bufs=cfg["misc_bufs"],
                                name="psv")
                for u in range(2):
                    j = g * 2 + u
                    nc.tensor.matmul(
                        psv[:, u * 256:(u + 1) * 256],
                        lhsT=kvx_sb[:, j * 128:(j + 1) * 128], rhs=wv2_sb,
                        start=True, stop=True,
                    )
                psv_v = psv.rearrange("p (g j c) -> p g j c", g=2, j=2)[:, :, 0, :]
                dst_v = v_sb[:, g * 256:(g + 1) * 256].rearrange(
                    "p (g c) -> p g c", g=2
                )
                eng_copy(dst_v, psv_v)

            if not cfg["interleave"]:
                for i in range(M // 512):
                    emit_kproj(i)
                for g in range(M // 256):
                    emit_vproj(g)

            # ---- attention, streamed over m in chunks of 128, per half ----
            for h in range(NQ // 512):
                qs = qT[:, h * 512:(h + 1) * 512]
                pso = psum.tile([128, 512], F32, tag="ps_o", bufs=1)
                pssum = psum.tile([1, 512], F32, tag="ps_sum", bufs=1)
                acc_d = acc_g = None
                if sum_mode == "dve":
                    acc_d = owork.tile([128, 512], F32, tag="acc_d", bufs=1)
                    acc_g = owork.tile([128, 512], F32, tag="acc_g", bufs=1)
                for j in range(32):
                    if cfg["interleave"] and h == 0:
                        if j % 4 == 0:
                            emit_kproj(j // 4)
                        if j % 2 == 0:
                            emit_vproj(j // 2)
                    pss = psum.tile([128, 512], F32, tag="ps_s",
                                    bufs=cfg["ps_s_bufs"])
                    nc.tensor.matmul(
                        pss, lhsT=kT[:, j * 128:(j + 1) * 128], rhs=qs,
                        start=True, stop=True,
                    )
                    p_sb = pwork.tile([128, 512], F32R, tag="p_sb",
                                      bufs=cfg["p_bufs"])
                    nc.scalar.activation(p_sb, pss, AF.Exp, scale=SCALE)
                    nc.tensor.matmul(
                        pso, lhsT=v_sb[:, j * 128:(j + 1) * 128], rhs=p_sb,
                        start=(j == 0), stop=(j == 31),
                    )
                    if sum_mode == "pe":
                        nc.tensor.matmul(
                            pssum, lhsT=ones_col, rhs=p_sb,
                            start=(j == 0), stop=(j == 31),
                        )
                    else:
                        pf = p_sb.bitcast(F32)
                        on_gp = (j % cfg["gp_every"] == cfg["gp_every"] - 1)
                        if j == 0:
                            nc.vector.tensor_copy(acc_d, pf)
                        elif j == 1:
                            nc.gpsimd.tensor_copy(acc_g, pf)
                        elif on_gp:
                            nc.gpsimd.tensor_add(acc_g, acc_g, pf)
                        else:
                            nc.vector.tensor_add(acc_d, acc_d, pf)

                if sum_mode == "dve":
                    nc.vector.tensor_add(acc_d, acc_d, acc_g)
                    nc.tensor.matmul(pssum, lhsT=ones_f32, rhs=acc_d,
                                     start=True, stop=True)
                # normalize-during-output: r transposed per n-block via tiny
                # K=1 matmuls; scale applied in the post-transpose copy.
                r_row = owork.tile([1, 512], F32, tag="r_row")
                nc.vector.reciprocal(r_row, pssum)
                o_sb = owork.tile([128, 512], F32, tag="o_sb")
                nc.vector.tensor_copy(o_sb, pso)
                for nb in range(4):
                    psr = psum.tile([128, 512], F32, tag="ps_misc", bufs=cfg["misc_bufs"])
                    nc.tensor.matmul(
                        psr[:, 0:1], lhsT=r_row[:, nb * 128:(nb + 1) * 128],
                        rhs=ones_row[:, 0:1], start=True, stop=True,
                    )
                    r_col = outp.tile([128, 1], F32, tag="r_col")
                    nc.vector.tensor_copy(r_col, psr[:, 0:1])
                    pst = psum.tile([128, 512], F32, tag="ps_misc", bufs=cfg["misc_bufs"])
                    nc.tensor.transpose(
                        pst[:, 0:128], o_sb[:, nb * 128:(nb + 1) * 128], ident
                    )
                    ot = outp.tile([128, 128], F32, tag="ot")
                    nc.vector.tensor_scalar_mul(ot, pst[:, 0:128], r_col)
                    nc.sync.dma_start(
                        out[h * 512 + nb * 128: h * 512 + (nb + 1) * 128, :], ot
                    )
        if loop_cm is not None:
            loop_cm.__exit__(None, None, None)
    nc.compile()
    return nc


def _prepare_in_maps(query, key_value, Wq, bq, Wk, bk, Wv, bv):
    q = np.ascontiguousarray(np.asarray(query, dtype=np.float32))
    kv = np.asarray(key_value, dtype=np.float32)
    wqT = np.asarray(Wq, np.float32).T
    wkT = np.asarray(Wk, np.float32).T
    wvT = np.asarray(Wv, np.float32).T
    bq_ = np.ascontiguousarray(np.asarray(bq, np.float32).reshape(C, 1))
    kpack = {}
    for b in range(B):
        kvx = kv[:, b].reshape(T, C, NQ).transpose(1, 0, 2).reshape(C, M)
        kpack[b] = np.ascontiguousarray(np.concatenate([wkT, wvT, kvx], axis=1))
    in_maps = []
    for core in range(N_CORES):
        b, t = divmod(core, T)
        qpack = np.ascontiguousarray(
            np.concatenate([wqT, q[t, b].reshape(C, NQ)], axis=1)
        )
        in_maps.append({"qpack": qpack, "kpack": kpack[b], "bq": bq_})
    return in_maps


def _assemble(results, bv):
    full = np.empty((B, T * NQ, C), np.float32)
    for core in range(N_CORES):
        b, t = divmod(core, T)
        full[b, t * NQ:(t + 1) * NQ] = results[core]["out"]
    full += np.asarray(bv, np.float32)[None, None, :]
    return full


def kernel(query, key_value, Wq, bq, Wk, bk, Wv, bv, **run_kwargs):
    global _NC
    if _NC is None:
        _NC = build_nc()
    in_maps = _prepare_in_maps(query, key_value, Wq, bq, Wk, bk, Wv, bv)
    res = run_bass_kernel_spmd(_NC, in_maps, list(range(N_CORES)), **run_kwargs)
    out = _assemble(res.results, bv)
    if run_kwargs:
        return out, res
    return out
